# revision 8
# baseline (speedup 1.0000x reference)
"""Bass/Tile kernel for nn_PostProModel on 8 Trainium2 NeuronCores.

Per batch element b (65536 total, data-parallel over 8 cores):
  x      = [prob[b] | member_idx]          [16, 4]
  hidden = relu(x @ W1 + b1)               [16, 128]
  S      = hidden @ hidden^T               [16, 16]
  A      = softmax(S, axis=-1)
  out    = (A @ hidden) @ W2 + b2          [16, 3]

Device mapping (per core: 8192 elements = 131072 (b,m) columns):
  - hiddenT [128h, cols] via one matmul per 512-col strip (lhsT=W1, rhs=probT_aug).
  - gram per 128-col pass (8 elems): S_blk = hT_p^T @ hT_p  (full [128,128],
    only the 8 diagonal 16x16 blocks are used downstream).
  - E = exp(S_blk) * block_mask  (multiplicative mask zeroes cross-element terms).
  - G'' = hT_p^T @ [W2|0] + [b2|1]  -> mmOut: lhsT=E (symmetric!), rhs=G''
    gives out_raw[.,0:3] and the softmax denominator in col 3 (ones column).
  - out = out_raw[:,0:3] * (1/denom)  (b2 already folded via G'').
Host does the layout transposes (cheap numpy) on both ends.
"""

import sys
import numpy as np

sys.path.insert(0, "/opt/trn_rl_repo")

N_CORES = 8
B_TOTAL, M, C, H = 65536, 16, 3, 128
B_CORE = B_TOTAL // N_CORES            # 8192
COLS = B_CORE * M                      # 131072
NB = 512                               # columns per strip (32 elements)
NPASS = 4                              # 128-col passes per strip
SUPER = 16                             # strips per output DMA flush
NSTRIP = COLS // NB                    # 256
NSUPER = NSTRIP // SUPER               # 16
OUT_W = 12 * SUPER                     # 192 output cols per staging tile

_CACHE = {}


def _build(nstrip, use_bf16=True):
    import concourse.bacc as bacc
    import concourse.tile as tile
    from concourse import mybir

    f32 = mybir.dt.float32
    DT = mybir.dt.bfloat16 if use_bf16 else f32  # hiddenT / E / weights dtype
    Alu = mybir.AluOpType
    Act = mybir.ActivationFunctionType

    nsuper = max(1, nstrip // SUPER)

    nc = bacc.Bacc("TRN2")
    probT = nc.dram_tensor("probT", [4, COLS], DT, kind="ExternalInput")
    w1 = nc.dram_tensor("w1", [4, H], DT, kind="ExternalInput")
    b1 = nc.dram_tensor("b1", [H, 1], f32, kind="ExternalInput")
    w2z = nc.dram_tensor("w2z", [H, 4], DT, kind="ExternalInput")
    c24 = nc.dram_tensor("c24", [H, 4], f32, kind="ExternalInput")
    m01 = nc.dram_tensor("m01", [128, 128], DT, kind="ExternalInput")
    outb = nc.dram_tensor("outb", [nsuper, 128, OUT_W], f32, kind="ExternalOutput")

    with tile.TileContext(nc) as tc:
        from contextlib import ExitStack
        with ExitStack() as ctx:
            singles = ctx.enter_context(tc.tile_pool(name="singles", bufs=1))
            px = ctx.enter_context(tc.tile_pool(name="px", bufs=4))
            ph = ctx.enter_context(tc.tile_pool(name="ph", bufs=3))
            pe = ctx.enter_context(tc.tile_pool(name="pe", bufs=3))
            pe2 = ctx.enter_context(tc.tile_pool(name="pe2", bufs=3))
            pg = ctx.enter_context(tc.tile_pool(name="pg", bufs=4))
            pr = ctx.enter_context(tc.tile_pool(name="pr", bufs=4))
            pout = ctx.enter_context(tc.tile_pool(name="pout", bufs=2))
            pH = ctx.enter_context(tc.tile_pool(name="pH", bufs=3, space="PSUM"))
            pS = ctx.enter_context(tc.tile_pool(name="pS", bufs=3, space="PSUM"))
            pGO = ctx.enter_context(tc.tile_pool(name="pGO", bufs=2, space="PSUM"))

            w1_t = singles.tile([4, H], DT)
            nc.sync.dma_start(out=w1_t, in_=w1[:, :])
            b1_t = singles.tile([H, 1], f32)
            nc.sync.dma_start(out=b1_t, in_=b1[:, :])
            w2z_t = singles.tile([H, 4], DT)
            nc.sync.dma_start(out=w2z_t, in_=w2z[:, :])
            c24_t = singles.tile([H, 4], f32)
            nc.sync.dma_start(out=c24_t, in_=c24[:, :])
            m01_t = singles.tile([128, 128], DT)
            nc.sync.dma_start(out=m01_t, in_=m01[:, :])

            outS = None
            for s in range(nstrip):
                su, t = divmod(s, SUPER)
                # --- stage 1: hiddenT strip [128h, 512 cols] ---
                xT = px.tile([4, NB], DT, tag="xT")
                nc.sync.dma_start(out=xT, in_=probT[:, NB * s:NB * (s + 1)])
                psumH = pH.tile([128, NB], f32, tag="psumH")
                nc.tensor.matmul(psumH[:, :], w1_t[:, :], xT[:, :],
                                 start=True, stop=True)
                hT = ph.tile([128, NB], DT, tag="hT")
                if s % 8 < 5:
                    nc.scalar.activation(hT[:, :], psumH[:, :], Act.Relu,
                                         bias=b1_t[:, 0:1], scale=1.0)
                else:
                    nc.vector.tensor_scalar(hT[:, :], psumH[:, :],
                                            scalar1=b1_t[:, 0:1], scalar2=0.0,
                                            op0=Alu.add, op1=Alu.max)

                # --- stage 2: gram + G per 128-col pass ---
                psumS = pS.tile([128, NB], f32, tag="psumS")
                psumGO = pGO.tile([128, 32], f32, tag="psumGO")
                psumG = psumGO[:, 0:16]
                for p in range(NPASS):
                    sl = hT[:, 128 * p:128 * (p + 1)]
                    nc.tensor.matmul(psumS[:, 128 * p:128 * (p + 1)], sl, sl,
                                     start=True, stop=True)
                    nc.tensor.matmul(psumG[:, 4 * p:4 * (p + 1)], sl,
                                     w2z_t[:, :], start=True, stop=True)

                # --- stage 3: E = exp(S) * mask ---
                E0 = pe.tile([128, NB], DT, tag="E0")
                nc.scalar.activation(E0[:, :], psumS[:, :], Act.Exp)
                E = pe2.tile([128, NB], DT, tag="E")
                e0v = E0[:, :].rearrange("q (g n) -> q g n", g=NPASS)
                ev = E[:, :].rearrange("q (g n) -> q g n", g=NPASS)
                mv = m01_t[:, None, :].broadcast_to([128, NPASS, 128])
                if s % 4 == 3:
                    nc.gpsimd.tensor_tensor(ev, e0v, mv, op=Alu.mult)
                else:
                    nc.vector.tensor_tensor(ev, e0v, mv, op=Alu.mult)

                # --- G'' = psumG + [b2|1] ---
                Gpp = pg.tile([128, 16], DT, tag="Gpp")
                gv = Gpp[:, :].rearrange("q (g c) -> q g c", g=NPASS)
                nc.vector.tensor_tensor(
                    gv, psumG[:, :].rearrange("q (g c) -> q g c", g=NPASS),
                    c24_t[:, None, :].broadcast_to([128, NPASS, 4]), op=Alu.add)

                # --- stage 4: out_raw + denom via lhsT=E (symmetric) ---
                psumO = psumGO[:, 16:32]
                for p in range(NPASS):
                    nc.tensor.matmul(psumO[:, 4 * p:4 * (p + 1)],
                                     E[:, 128 * p:128 * (p + 1)],
                                     Gpp[:, 4 * p:4 * (p + 1)],
                                     start=True, stop=True)

                # --- stage 5: divide by denom, stage out ---
                rden = pr.tile([128, NPASS], f32, tag="rden")
                pov = psumO[:, :].rearrange("q (g c) -> q g c", g=NPASS)
                nc.vector.reciprocal(rden[:, :], pov[:, :, 3])
                if t == 0:
                    outS = pout.tile([128, OUT_W], f32, tag="outS")
                ov = outS[:, 12 * t:12 * (t + 1)].rearrange(
                    "q (g c) -> q g c", g=NPASS)
                nc.vector.tensor_tensor(
                    ov, pov[:, :, 0:3],
                    rden[:, :, None].broadcast_to([128, NPASS, 3]), op=Alu.mult)
                if t == SUPER - 1 or s == nstrip - 1:
                    nc.sync.dma_start(out=outb[su, :, :], in_=outS[:, :])

    nc.finalize()
    return nc


def _prep_core_inputs(prob_core, W1, b1, W2, b2, use_bf16=True):
    pT = np.ascontiguousarray(prob_core.reshape(-1, C).T)        # [3, COLS]
    idx = np.tile(np.arange(M, dtype=np.float32), B_CORE)[None]  # [1, COLS]
    probT_aug = np.ascontiguousarray(np.concatenate([pT, idx], axis=0))
    w2z = np.concatenate([W2, np.zeros((H, 1), np.float32)], axis=1)
    c24 = np.concatenate([np.broadcast_to(b2[None, :], (H, C)),
                          np.ones((H, 1), np.float32)], axis=1)
    m01 = np.kron(np.eye(8, dtype=np.float32),
                  np.ones((16, 16), np.float32))
    import ml_dtypes
    ddt = ml_dtypes.bfloat16 if use_bf16 else np.float32
    return {
        "probT": probT_aug.astype(ddt),
        "w1": np.ascontiguousarray(W1.astype(np.float32)).astype(ddt),
        "b1": np.ascontiguousarray(b1.astype(np.float32).reshape(H, 1)),
        "w2z": np.ascontiguousarray(w2z.astype(np.float32)).astype(ddt),
        "c24": np.ascontiguousarray(c24),
        "m01": np.ascontiguousarray(m01).astype(ddt),
    }


def _postprocess(outb_arr):
    # outb [nsuper, 128, OUT_W]; partition q=(b_l*16+m); col = t*12 + p*3 + c
    nsuper = outb_arr.shape[0]
    ob = outb_arr.reshape(nsuper, 8, 16, SUPER, NPASS, 3)  # u, b_l, m, t, p, c
    ob = ob.transpose(0, 3, 4, 1, 2, 5)                    # u, t, p, b_l, m, c
    return np.ascontiguousarray(ob.reshape(nsuper * SUPER * NPASS * 8, M, C))


def _maybe_patch_ldwopt():
    import os
    if os.environ.get("PPK_LDWOPT") != "1":
        return
    import concourse.bass_utils as bu
    if getattr(bu, "_ppk_ldw_patched", False):
        return
    orig = bu.run_command

    def patched(argv, **kw):
        argv = [a.replace("--enable-ldw-opt=false", "--enable-ldw-opt=true")
                if isinstance(a, str) else a for a in argv]
        return orig(argv, **kw)

    bu.run_command = patched
    bu._ppk_ldw_patched = True


def kernel(prob, W1, b1, W2, b2, _trace=False):
    import os
    from concourse.bass_utils import run_bass_kernel_spmd
    _maybe_patch_ldwopt()

    nstrip = int(os.environ.get("PPK_NSTRIP", NSTRIP))
    use_bf16 = os.environ.get("PPK_DTYPE", "bf16") == "bf16"
    if "nc" not in _CACHE:
        _CACHE["nc"] = _build(nstrip, use_bf16)
    nc = _CACHE["nc"]

    prob = np.asarray(prob, np.float32)
    in_maps = []
    for ci in range(N_CORES):
        pc = prob[ci * B_CORE:(ci + 1) * B_CORE]
        in_maps.append(_prep_core_inputs(pc, np.asarray(W1), np.asarray(b1),
                                         np.asarray(W2), np.asarray(b2), use_bf16))
    res = run_bass_kernel_spmd(nc, in_maps, list(range(N_CORES)),
                               trace=_trace)
    _CACHE["last_result"] = res
    out = np.zeros((B_TOTAL, M, C), np.float32)
    for ci in range(N_CORES):
        o = _postprocess(res.results[ci]["outb"])
        out[ci * B_CORE:ci * B_CORE + o.shape[0]] = o
    return out


# revision 9
# speedup vs baseline: 1.2067x; 1.2067x over previous
"""Bass/Tile kernel for nn_PostProModel on 8 Trainium2 NeuronCores.

Per batch element b (65536 total, data-parallel over 8 cores):
  x      = [prob[b] | member_idx]          [16, 4]
  hidden = relu(x @ W1 + b1)               [16, 128]
  S      = hidden @ hidden^T               [16, 16]
  A      = softmax(S, axis=-1)
  out    = (A @ hidden) @ W2 + b2          [16, 3]

Device mapping (per core: 8192 elements = 131072 (b,m) columns):
  - hiddenT [128h, cols] via one matmul per 512-col strip (lhsT=W1, rhs=probT_aug).
  - gram per 128-col pass (8 elems): S_blk = hT_p^T @ hT_p  (full [128,128],
    only the 8 diagonal 16x16 blocks are used downstream).
  - E = exp(S_blk) * block_mask  (multiplicative mask zeroes cross-element terms).
  - G'' = hT_p^T @ [W2|0] + [b2|1]  -> mmOut: lhsT=E (symmetric!), rhs=G''
    gives out_raw[.,0:3] and the softmax denominator in col 3 (ones column).
  - out = out_raw[:,0:3] * (1/denom)  (b2 already folded via G'').
Host does the layout transposes (cheap numpy) on both ends.
"""

import sys
import numpy as np

sys.path.insert(0, "/opt/trn_rl_repo")

N_CORES = 8
B_TOTAL, M, C, H = 65536, 16, 3, 128
B_CORE = B_TOTAL // N_CORES            # 8192
COLS = B_CORE * M                      # 131072
NB = 512                               # columns per strip (32 elements)
NPASS = 4                              # 128-col passes per strip
SUPER = 16                             # strips per output DMA flush
NSTRIP = COLS // NB                    # 256
NSUPER = NSTRIP // SUPER               # 16
OUT_W = 12 * SUPER                     # 192 output cols per staging tile

_CACHE = {}


def _build(nstrip, use_bf16=True):
    import concourse.bacc as bacc
    import concourse.tile as tile
    from concourse import mybir

    f32 = mybir.dt.float32
    DT = mybir.dt.bfloat16 if use_bf16 else f32  # hiddenT / E / weights dtype
    Alu = mybir.AluOpType
    Act = mybir.ActivationFunctionType

    nsuper = max(1, nstrip // SUPER)

    nc = bacc.Bacc("TRN2")
    probT = nc.dram_tensor("probT", [4, COLS], DT, kind="ExternalInput")
    w1 = nc.dram_tensor("w1", [4, H], DT, kind="ExternalInput")
    b1 = nc.dram_tensor("b1", [H, 1], f32, kind="ExternalInput")
    w2z = nc.dram_tensor("w2z", [H, 4], DT, kind="ExternalInput")
    c24 = nc.dram_tensor("c24", [H, 4], f32, kind="ExternalInput")
    m01 = nc.dram_tensor("m01", [128, 128], DT, kind="ExternalInput")
    outb = nc.dram_tensor("outb", [nsuper, 128, OUT_W], f32, kind="ExternalOutput")

    with tile.TileContext(nc) as tc:
        from contextlib import ExitStack
        with ExitStack() as ctx:
            singles = ctx.enter_context(tc.tile_pool(name="singles", bufs=1))
            px = ctx.enter_context(tc.tile_pool(name="px", bufs=4))
            ph = ctx.enter_context(tc.tile_pool(name="ph", bufs=3))
            pe = ctx.enter_context(tc.tile_pool(name="pe", bufs=3))
            pe2 = ctx.enter_context(tc.tile_pool(name="pe2", bufs=3))
            pg = ctx.enter_context(tc.tile_pool(name="pg", bufs=4))
            pr = ctx.enter_context(tc.tile_pool(name="pr", bufs=4))
            pout = ctx.enter_context(tc.tile_pool(name="pout", bufs=2))
            pH = ctx.enter_context(tc.tile_pool(name="pH", bufs=3, space="PSUM"))
            pS = ctx.enter_context(tc.tile_pool(name="pS", bufs=3, space="PSUM"))
            pGO = ctx.enter_context(tc.tile_pool(name="pGO", bufs=2, space="PSUM"))

            w1_t = singles.tile([4, H], DT)
            nc.sync.dma_start(out=w1_t, in_=w1[:, :])
            b1_t = singles.tile([H, 1], f32)
            nc.sync.dma_start(out=b1_t, in_=b1[:, :])
            w2z_t = singles.tile([H, 4], DT)
            nc.sync.dma_start(out=w2z_t, in_=w2z[:, :])
            c24_t = singles.tile([H, 4], f32)
            nc.sync.dma_start(out=c24_t, in_=c24[:, :])
            m01_t = singles.tile([128, 128], DT)
            nc.sync.dma_start(out=m01_t, in_=m01[:, :])

            outS = None
            for s in range(nstrip):
                su, t = divmod(s, SUPER)
                # --- stage 1: hiddenT strip [128h, 512 cols] ---
                xT = px.tile([4, NB], DT, tag="xT")
                nc.sync.dma_start(out=xT, in_=probT[:, NB * s:NB * (s + 1)])
                psumH = pH.tile([128, NB], f32, tag="psumH")
                nc.tensor.matmul(psumH[:, :], w1_t[:, :], xT[:, :],
                                 start=True, stop=True)
                hT = ph.tile([128, NB], DT, tag="hT")
                if s % 2 == 0:
                    nc.scalar.activation(hT[:, :], psumH[:, :], Act.Relu,
                                         bias=b1_t[:, 0:1], scale=1.0)
                else:
                    nc.vector.tensor_scalar(hT[:, :], psumH[:, :],
                                            scalar1=b1_t[:, 0:1], scalar2=0.0,
                                            op0=Alu.add, op1=Alu.max)

                # --- stage 2: gram + G per 128-col pass ---
                psumS = pS.tile([128, NB], f32, tag="psumS")
                psumGO = pGO.tile([128, 32], f32, tag="psumGO")
                psumG = psumGO[:, 0:16]
                for p in range(NPASS):
                    sl = hT[:, 128 * p:128 * (p + 1)]
                    nc.tensor.matmul(psumS[:, 128 * p:128 * (p + 1)], sl, sl,
                                     start=True, stop=True)
                    nc.tensor.matmul(psumG[:, 4 * p:4 * (p + 1)], sl,
                                     w2z_t[:, :], start=True, stop=True)

                # --- stage 3: E = exp(S) * mask ---
                E0 = pe.tile([128, NB], DT, tag="E0")
                nc.scalar.activation(E0[:, :], psumS[:, :], Act.Exp)
                E = pe2.tile([128, NB], DT, tag="E")
                e0v = E0[:, :].rearrange("q (g n) -> q g n", g=NPASS)
                ev = E[:, :].rearrange("q (g n) -> q g n", g=NPASS)
                mv = m01_t[:, None, :].broadcast_to([128, NPASS, 128])
                if s % 2 == 0:
                    nc.vector.tensor_tensor(ev, e0v, mv, op=Alu.mult)
                else:
                    nc.gpsimd.tensor_tensor(ev, e0v, mv, op=Alu.mult)

                # --- G'' = psumG + [b2|1] ---
                Gpp = pg.tile([128, 16], DT, tag="Gpp")
                gv = Gpp[:, :].rearrange("q (g c) -> q g c", g=NPASS)
                nc.vector.tensor_tensor(
                    gv, psumG[:, :].rearrange("q (g c) -> q g c", g=NPASS),
                    c24_t[:, None, :].broadcast_to([128, NPASS, 4]), op=Alu.add)

                # --- stage 4: out_raw + denom via lhsT=E (symmetric) ---
                psumO = psumGO[:, 16:32]
                for p in range(NPASS):
                    nc.tensor.matmul(psumO[:, 4 * p:4 * (p + 1)],
                                     E[:, 128 * p:128 * (p + 1)],
                                     Gpp[:, 4 * p:4 * (p + 1)],
                                     start=True, stop=True)

                # --- stage 5: divide by denom, stage out ---
                rden = pr.tile([128, NPASS], f32, tag="rden")
                pov = psumO[:, :].rearrange("q (g c) -> q g c", g=NPASS)
                nc.vector.reciprocal(rden[:, :], pov[:, :, 3])
                if t == 0:
                    outS = pout.tile([128, OUT_W], f32, tag="outS")
                ov = outS[:, 12 * t:12 * (t + 1)].rearrange(
                    "q (g c) -> q g c", g=NPASS)
                nc.vector.tensor_tensor(
                    ov, pov[:, :, 0:3],
                    rden[:, :, None].broadcast_to([128, NPASS, 3]), op=Alu.mult)
                if t == SUPER - 1 or s == nstrip - 1:
                    nc.sync.dma_start(out=outb[su, :, :], in_=outS[:, :])

    nc.finalize()
    return nc


def _prep_core_inputs(prob_core, W1, b1, W2, b2, use_bf16=True):
    pT = np.ascontiguousarray(prob_core.reshape(-1, C).T)        # [3, COLS]
    idx = np.tile(np.arange(M, dtype=np.float32), B_CORE)[None]  # [1, COLS]
    probT_aug = np.ascontiguousarray(np.concatenate([pT, idx], axis=0))
    w2z = np.concatenate([W2, np.zeros((H, 1), np.float32)], axis=1)
    c24 = np.concatenate([np.broadcast_to(b2[None, :], (H, C)),
                          np.ones((H, 1), np.float32)], axis=1)
    m01 = np.kron(np.eye(8, dtype=np.float32),
                  np.ones((16, 16), np.float32))
    import ml_dtypes
    ddt = ml_dtypes.bfloat16 if use_bf16 else np.float32
    return {
        "probT": probT_aug.astype(ddt),
        "w1": np.ascontiguousarray(W1.astype(np.float32)).astype(ddt),
        "b1": np.ascontiguousarray(b1.astype(np.float32).reshape(H, 1)),
        "w2z": np.ascontiguousarray(w2z.astype(np.float32)).astype(ddt),
        "c24": np.ascontiguousarray(c24),
        "m01": np.ascontiguousarray(m01).astype(ddt),
    }


def _postprocess(outb_arr):
    # outb [nsuper, 128, OUT_W]; partition q=(b_l*16+m); col = t*12 + p*3 + c
    nsuper = outb_arr.shape[0]
    ob = outb_arr.reshape(nsuper, 8, 16, SUPER, NPASS, 3)  # u, b_l, m, t, p, c
    ob = ob.transpose(0, 3, 4, 1, 2, 5)                    # u, t, p, b_l, m, c
    return np.ascontiguousarray(ob.reshape(nsuper * SUPER * NPASS * 8, M, C))


def _maybe_patch_ldwopt():
    import os
    if os.environ.get("PPK_LDWOPT") != "1":
        return
    import concourse.bass_utils as bu
    if getattr(bu, "_ppk_ldw_patched", False):
        return
    orig = bu.run_command

    def patched(argv, **kw):
        argv = [a.replace("--enable-ldw-opt=false", "--enable-ldw-opt=true")
                if isinstance(a, str) else a for a in argv]
        return orig(argv, **kw)

    bu.run_command = patched
    bu._ppk_ldw_patched = True


def kernel(prob, W1, b1, W2, b2, _trace=False):
    import os
    from concourse.bass_utils import run_bass_kernel_spmd
    _maybe_patch_ldwopt()

    nstrip = int(os.environ.get("PPK_NSTRIP", NSTRIP))
    use_bf16 = os.environ.get("PPK_DTYPE", "bf16") == "bf16"
    if "nc" not in _CACHE:
        _CACHE["nc"] = _build(nstrip, use_bf16)
    nc = _CACHE["nc"]

    prob = np.asarray(prob, np.float32)
    in_maps = []
    for ci in range(N_CORES):
        pc = prob[ci * B_CORE:(ci + 1) * B_CORE]
        in_maps.append(_prep_core_inputs(pc, np.asarray(W1), np.asarray(b1),
                                         np.asarray(W2), np.asarray(b2), use_bf16))
    res = run_bass_kernel_spmd(nc, in_maps, list(range(N_CORES)),
                               trace=_trace)
    _CACHE["last_result"] = res
    out = np.zeros((B_TOTAL, M, C), np.float32)
    for ci in range(N_CORES):
        o = _postprocess(res.results[ci]["outb"])
        out[ci * B_CORE:ci * B_CORE + o.shape[0]] = o
    return out
